# revision 1
# baseline (speedup 1.0000x reference)
"""Causal attention (B=4, S=2048, D=1024, fp32) on 8 TRN2 NeuronCores.

Sharding: core c -> (batch b = c//2, key-parity h = c%2). Each core computes
q = x@Wq.T for all S queries of its batch, k/v only for key positions whose
128-block index has parity h (S/2 positions, causally load-balanced), then
scores^T = k q^T in [kpos, q] orientation (softmax denominator and A@V both
reduce over kpos = the PSUM contraction dim, so no on-chip transposes), and
returns the unnormalized partial output sum(exp(s)*v) plus the denominator
sum(exp(s)). Host adds the two partials per batch and divides. exp() is
computed without max-subtraction: scores*scale is ~N(0, 0.17) here, far from
fp32 overflow. All matmuls run as float32r (fp32 truncated inside the PE),
which streams at ~1 col/cycle warm for moving dims >= 256.
"""
import numpy as np

import concourse.bacc as bacc
import concourse.tile as tile
import concourse.mybir as mybir
from concourse import bass_utils
from concourse.tile import add_dep_helper
from contextlib import ExitStack

B, S, D = 4, 2048, 1024
QT = 256              # query tile
NT = S // QT          # 8 query tiles
SH = S // 2           # key positions per core
SCALE = 1.0 / 32.0    # 1/sqrt(D)
F32 = mybir.dt.float32
F32R = mybir.dt.float32r
EXP = mybir.ActivationFunctionType.Exp

_NC = None


def _dview(ap):
    """[D, C] dram tensor -> [128, 8, C] view (partition, d-block, col)."""
    return ap.rearrange("(d p) c -> p d c", p=128)


def _build():
    nc = bacc.Bacc()
    xT = nc.dram_tensor("xT", [D, S], F32, kind="ExternalInput").ap()
    xkT = nc.dram_tensor("xkT", [D, SH], F32, kind="ExternalInput").ap()
    wqT = nc.dram_tensor("wqT", [D, D], F32, kind="ExternalInput").ap()
    wkT = nc.dram_tensor("wkT", [D, D], F32, kind="ExternalInput").ap()
    wvT = nc.dram_tensor("wvT", [D, D], F32, kind="ExternalInput").ap()
    dmask = nc.dram_tensor("dmask", [2, 128, 512], F32, kind="ExternalInput").ap()
    pout = nc.dram_tensor("pout", [S, D], F32, kind="ExternalOutput").ap()
    den = nc.dram_tensor("den", [128, 2 * NT], F32, kind="ExternalOutput").ap()

    def chain_to(inst, prev):
        add_dep_helper(inst.ins, prev.ins, sync=True, reason="input dma ordering")
        return inst

    with tile.TileContext(nc) as tc, ExitStack() as top:
        small = top.enter_context(tc.tile_pool(name="small", bufs=1))
        osb_pool = top.enter_context(tc.tile_pool(name="osb", bufs=2))
        qt_pool = top.enter_context(tc.tile_pool(name="qt", bufs=1))

        qt = [qt_pool.tile([128, S], F32R, tag=f"qt{e}", name=f"qt{e}") for e in range(8)]
        ones_f = small.tile([128, 2], F32)
        ones = small.tile([128, 2], F32R)
        den_acc = small.tile([128, 2 * NT], F32)
        junk = small.tile([128, 512], F32R)
        nc.vector.memset(ones_f, 1.0)
        nc.vector.tensor_copy(ones, ones_f)
        nc.vector.memset(junk.bitcast(F32), 0.0)
        nc.vector.tensor_copy(junk, junk)

        # ---- phase 1: q^T = Wq^T-contracted x^T, for all S queries ----
        # Emitted first; the first PE group depends only on a 0.5 MiB Wq
        # e0-slice plus one 1 MiB x^T chunk. Everything else is chained
        # behind those so HBM bandwidth follows consumption order.
        attn = top.enter_context(ExitStack())
        with ExitStack() as ph:
            warm_ps = ph.enter_context(tc.tile_pool(name="warm", bufs=1, space="PSUM"))
            wp = warm_ps.tile([128, 512], F32, name="wp")
            for _ in range(30):
                nc.tensor.matmul(wp[0:2, :], lhsT=junk[:, 0:2], rhs=junk,
                                 start=True, stop=True, skip_group_check=True)
            wq_pool = ph.enter_context(tc.tile_pool(name="wq", bufs=1))
            xs_pool = ph.enter_context(tc.tile_pool(name="xs", bufs=2))
            psB = ph.enter_context(tc.tile_pool(name="psB", bufs=4, space="PSUM"))
            wq = wq_pool.tile([128, 8, D], F32R, name="wq")
            wqv = _dview(wqT.bitcast(F32R))
            xv = _dview(xT.bitcast(F32R))
            nc.sync.dma_start(out=wq[:, :, 0:128], in_=wqv[:, :, 0:128])
            # 512-wide q chunks (so the matmul stream, 213ns, hides the
            # 2-pass fp32r LDWEIGHTS ~195ns), loaded as two 1 MiB half-DMAs
            # for fine arrival granularity.
            xs_dma = [None] * (S // 512)
            xs_tiles = [None] * (S // 512)
            xs_tiles[0] = xs_pool.tile([128, 8, 512], F32R, name="xs")
            nc.sync.dma_start(out=xs_tiles[0][:, :, 0:256], in_=xv[:, :, 0:256])
            xs_dma[0] = nc.sync.dma_start(out=xs_tiles[0][:, :, 256:512], in_=xv[:, :, 256:512])
            wq1b = nc.sync.dma_start(out=wq[:, :, 128:256], in_=wqv[:, :, 128:256])
            wq2 = nc.sync.dma_start(out=wq[:, :, 256:512], in_=wqv[:, :, 256:512])
            wq3 = chain_to(nc.sync.dma_start(out=wq[:, :, 512:D], in_=wqv[:, :, 512:D]), wq1b)
            for c in range(S // 512):
                if c > 0:
                    xs_tiles[c] = xs_pool.tile([128, 8, 512], F32R, name="xs")
                    prev = wq2 if c == 1 else xs_dma[c - 1]
                    chain_to(nc.sync.dma_start(
                        out=xs_tiles[c][:, :, 0:256],
                        in_=xv[:, :, c * 512:c * 512 + 256]), prev)
                    xs_dma[c] = chain_to(nc.sync.dma_start(
                        out=xs_tiles[c][:, :, 256:512],
                        in_=xv[:, :, c * 512 + 256:(c + 1) * 512]), prev)
                xs = xs_tiles[c]
                for e in range(8):
                    ps = psB.tile([128, 512], F32)
                    for d_ in range(8):
                        nc.tensor.matmul(ps, lhsT=wq[:, d_, e * 128:(e + 1) * 128],
                                         rhs=xs[:, d_, :], start=d_ == 0, stop=d_ == 7)
                    nc.vector.tensor_copy(qt[e][:, c * 512:(c + 1) * 512], ps)

        # ---- SBUF layout for the rest: kt+wv reuse the freed wq/xs space
        # (their fills naturally happen after the q^T phase); xk+wk go into
        # fresh space so their DMAs can land *during* the q^T phase. ----
        kt_pool = attn.enter_context(tc.tile_pool(name="kt", bufs=1))
        wv_pool = attn.enter_context(tc.tile_pool(name="wv", bufs=1))
        xk_pool = attn.enter_context(tc.tile_pool(name="xk", bufs=1))
        kt = [kt_pool.tile([128, SH], F32R, tag=f"kt{e}", name=f"kt{e}") for e in range(8)]
        wv = wv_pool.tile([128, 8, D], F32R, name="wv")
        xk = xk_pool.tile([128, 8, SH], F32R, name="xk")
        ixk = chain_to(nc.sync.dma_start(out=xk, in_=_dview(xkT.bitcast(F32R))), wq3)
        iwv = nc.sync.dma_start(out=wv, in_=_dview(wvT.bitcast(F32R)))

        with ExitStack() as ph:
            wk_pool = ph.enter_context(tc.tile_pool(name="wk", bufs=1))
            psA = ph.enter_context(tc.tile_pool(name="psA", bufs=4, space="PSUM"))
            wk = wk_pool.tile([128, 8, D], F32R, name="wk")
            iwk = chain_to(nc.sync.dma_start(out=wk, in_=_dview(wkT.bitcast(F32R))), ixk)
            add_dep_helper(iwv.ins, iwk.ins, sync=True, reason="input dma ordering")
            for sc in range(2):
                for e in range(8):
                    ps = psA.tile([128, 512], F32)
                    for d_ in range(8):
                        nc.tensor.matmul(
                            ps, lhsT=wk[:, d_, e * 128:(e + 1) * 128],
                            rhs=xk[:, d_, sc * 512:(sc + 1) * 512],
                            start=d_ == 0, stop=d_ == 7)
                    nc.vector.tensor_copy(kt[e][:, sc * 512:(sc + 1) * 512], ps)

        # ---- v projection into the space wk just freed ----
        v_pool = attn.enter_context(tc.tile_pool(name="v", bufs=1))
        vt = [v_pool.tile([128, D], F32R, tag=f"v{s}", name=f"v{s}") for s in range(8)]
        with ExitStack() as ph:
            psA2 = ph.enter_context(tc.tile_pool(name="psA2", bufs=4, space="PSUM"))
            for ec in range(2):
                for s_ in range(8):
                    ps = psA2.tile([128, 512], F32)
                    for d_ in range(8):
                        nc.tensor.matmul(
                            ps, lhsT=xk[:, d_, s_ * 128:(s_ + 1) * 128],
                            rhs=wv[:, d_, ec * 512:(ec + 1) * 512],
                            start=d_ == 0, stop=d_ == 7)
                    nc.vector.tensor_copy(vt[s_][:, ec * 512:(ec + 1) * 512], ps)

        # ---- attention over 512-query tiles, largest first. scores run at
        # N=512 so the 2-pass fp32r LDWEIGHTS stays hidden under the matmul
        # stream. A@V needs 8 psum banks for a 512-query out accumulation,
        # which does not fit next to the scores banks, so it runs as two
        # passes (q-halves) over the retained exp tiles. The dead xk tile is
        # reused as the exp-slot scratch and the dead wv tile holds eacc and
        # the diagonal masks.
        NU = S // 512
        dm_a = wv[:, 1, 0:512].bitcast(F32)
        dm_b = wv[:, 2, 0:512].bitcast(F32)
        nc.sync.dma_start(out=wv[:, 1, 0:512], in_=dmask[0].bitcast(F32R))
        nc.sync.dma_start(out=wv[:, 2, 0:512], in_=dmask[1].bitcast(F32R))
        ps_sc = attn.enter_context(tc.tile_pool(name="ps_sc", bufs=2, space="PSUM"))
        ps_out = attn.enter_context(tc.tile_pool(name="ps_out", bufs=1, space="PSUM"))
        ps_den = attn.enter_context(tc.tile_pool(name="ps_den", bufs=1, space="PSUM"))

        def av_pass(u, qs, jmax, eacc):
            """A@V + den + drain for q128 slices `qs`, k-blocks 0..jmax."""
            outp = [[ps_out.tile([128, 512], F32, tag=f"po{q & 1}{ec}", name=f"po{q & 1}{ec}")
                     for ec in range(2)] for q in qs]
            for jj in range(jmax + 1):
                for qi, q in enumerate(qs):
                    for ec in range(2):
                        nc.tensor.matmul(
                            outp[qi][ec], lhsT=xk[:, jj, q * 128:(q + 1) * 128],
                            rhs=vt[jj][:, ec * 512:(ec + 1) * 512],
                            start=jj == 0, stop=jj == jmax)
            for qi, q in enumerate(qs):
                denp = ps_den.tile([128, 2], F32, tag=f"pd{q & 1}", name=f"pd{q & 1}")
                nc.tensor.matmul(denp, lhsT=eacc[:, q * 128:(q + 1) * 128],
                                 rhs=ones, start=True, stop=True)
                row = u * 512 + q * 128
                osb = osb_pool.tile([128, D], F32, tag="osb", name="osb")
                nc.vector.tensor_copy(osb[:, 0:512], outp[qi][0])
                nc.scalar.copy(osb[:, 512:1024], outp[qi][1])
                nc.sync.dma_start(out=pout[row:row + 128, 0:512], in_=osb[:, 0:512])
                nc.sync.dma_start(out=pout[row:row + 128, 512:D], in_=osb[:, 512:D])
                nc.vector.tensor_copy(den_acc[:, 4 * u + q:4 * u + q + 1], denp[:, 0:1])

        for u in reversed(range(NU)):
            eacc = wv[:, 3 + (u & 1), 0:512]
            for jj in range(2 * u + 2):
                sp = ps_sc.tile([128, 512], F32)
                for e in range(8):
                    nc.tensor.matmul(
                        sp, lhsT=kt[e][:, jj * 128:(jj + 1) * 128],
                        rhs=qt[e][:, u * 512:(u + 1) * 512],
                        start=e == 0, stop=e == 7)
                if jj == 2 * u:
                    nc.vector.tensor_add(sp, sp, dm_a)
                elif jj == 2 * u + 1:
                    nc.vector.tensor_add(sp, sp, dm_b)
                et = xk[:, jj, 0:512]
                nc.scalar.activation(et, sp, EXP, scale=SCALE)
                if jj == 0:
                    nc.vector.tensor_copy(eacc, et)
                else:
                    nc.vector.tensor_add(eacc, eacc, et)
            av_pass(u, (0, 1), 2 * u, eacc)
            av_pass(u, (2, 3), 2 * u + 1, eacc)
        nc.sync.dma_start(out=den, in_=den_acc)

    nc.compile()
    return nc


def _prep_inputs(x, Wq, Wk, Wv):
    wqT = np.ascontiguousarray(Wq.T)
    wkT = np.ascontiguousarray(Wk.T)
    wvT = np.ascontiguousarray(Wv.T)
    i = np.arange(128)[:, None]
    j = np.arange(512)[None, :]
    in_maps = []
    for c in range(8):
        b, h = c // 2, c % 2
        xb = x[b]                                   # [S, D]
        xT = np.ascontiguousarray(xb.T)             # [D, S]
        xk = xb.reshape(S // 128, 128, D)[h::2].reshape(SH, D)
        xkT = np.ascontiguousarray(xk.T)            # [D, S/2]
        dm_a = np.where(j >= i + 128 * h, np.float32(0.0), np.float32(-1e30))
        dm_b = np.where(j >= 256 + i + 128 * h, np.float32(0.0), np.float32(-1e30))
        dmask = np.stack([dm_a, dm_b]).astype(np.float32)
        in_maps.append({
            "xT": xT, "xkT": xkT, "wqT": wqT, "wkT": wkT, "wvT": wvT,
            "dmask": np.ascontiguousarray(dmask),
        })
    return in_maps


def _run(inputs, trace=False, **kw):
    global _NC
    if _NC is None:
        _NC = _build()
    x = np.asarray(inputs["x"], dtype=np.float32)
    Wq = np.asarray(inputs["Wq"], dtype=np.float32)
    Wk = np.asarray(inputs["Wk"], dtype=np.float32)
    Wv = np.asarray(inputs["Wv"], dtype=np.float32)
    in_maps = _prep_inputs(x, Wq, Wk, Wv)
    res = bass_utils.run_bass_kernel_spmd(
        _NC, in_maps, core_ids=list(range(8)), trace=trace, **kw)
    out = np.empty((B, S, D), dtype=np.float32)
    for b in range(B):
        po = res.results[2 * b]["pout"] + res.results[2 * b + 1]["pout"]
        dn = res.results[2 * b]["den"] + res.results[2 * b + 1]["den"]
        out[b] = po / dn.T.reshape(S, 1)
    return out, res


def kernel(**inputs):
    out, _ = _run(inputs, trace=False)
    return out

